# revision 1
# baseline (speedup 1.0000x reference)
"""Max-pooling over sequence spans — Trainium2 Bass kernel.

Problem: context [B=8, S=4096, H=1024] f32; spans_begin/spans_len [B, 100] i32.
Output [B, 100, H] f32: out[b, n] = max over rows context[b, begin:begin+max(len,1)].

Sharding: pure data-parallel over the batch axis — one batch row per
NeuronCore, 8 cores, no cross-device communication.

Per-core algorithm (bit-exact f32):
  * Host precomputes gather indices idx[n, l] = begin[n] + min(l, eff_len[n]-1)
    for l in [0, 64).  Steps beyond a span's length re-read its last valid row,
    which is a no-op under max, so no masking is needed on device.
  * Device runs 64 accumulation steps.  Step l issues one indirect DMA
    (gpsimd SWDGE) that gathers row idx[n, l] for every span n into a
    [100, 1024] SBUF slab — one 4 KiB descriptor per span — and the vector
    engine folds the slab into one of two rotating accumulators with
    elementwise f32 max.  Two accumulator chains + 6 slab buffers keep the
    DMA stream and DVE fully overlapped; DVE (fp32 tensor_tensor max runs at
    1 elem/cycle/lane) is the ~70 us bottleneck, with the ~25 MB gather
    stream hidden under it.
  * The two accumulators are folded and written back to DRAM.

kernel() compiles the Bass program on first call (~1 s) and caches it for
the lifetime of the process.  It is value-generic: all span data flows in
as device tensors, so any inputs of the declared shapes work.
"""

import sys
import numpy as np

sys.path.insert(0, "/opt/trn_rl_repo")

B, S, H = 8, 4096, 1024
N_SPANS = 100
MAX_LEN = 64
N_CORES = 8

_cache = {}


def _build_program(n_steps, k_bufs, repeat=1, acc_bf16=False, n_slab_bufs=6,
                   gp_memset=False):
    """Build + compile the per-core SPMD program.

    repeat: replicate the kernel body (used only by timing harnesses;
    repeat-delta isolates per-iteration HW time from call overhead).
    acc_bf16: accumulate in bf16 (2x DVE rate, ~4e-3 rel err) — not used by
    kernel(); kept for experimentation.
    """
    import concourse.bass as bass
    import concourse.bacc as bacc
    import concourse.mybir as mybir
    import concourse.tile as tile

    acc_dt = mybir.dt.bfloat16 if acc_bf16 else mybir.dt.float32

    nc = bacc.Bacc("TRN2", target_bir_lowering=False, debug=False,
                   num_devices=N_CORES)
    ctx_d = nc.dram_tensor("ctx", [S, H], mybir.dt.float32, kind="ExternalInput")
    idx_d = nc.dram_tensor("idx", [N_SPANS, n_steps], mybir.dt.int32,
                           kind="ExternalInput")
    out_d = nc.dram_tensor("out", [N_SPANS, H], mybir.dt.float32,
                           kind="ExternalOutput")

    with tile.TileContext(nc) as tc:
        with (
            tc.tile_pool(name="persist", bufs=1) as persist,
            tc.tile_pool(name="slabs", bufs=n_slab_bufs) as slabs,
        ):
            idx_t = persist.tile([N_SPANS, n_steps], mybir.dt.int32)
            nc.sync.dma_start(out=idx_t[:], in_=idx_d[:])
            for _ in range(repeat):
                accs = []
                for k in range(k_bufs):
                    acc = persist.tile([N_SPANS, H], acc_dt, tag=f"acc{k}")
                    (nc.gpsimd if gp_memset else nc.vector).memset(acc[:],
                                                                   -3.0e38)
                    accs.append(acc)
                for l in range(n_steps):
                    slab = slabs.tile([N_SPANS, H], acc_dt)
                    nc.gpsimd.indirect_dma_start(
                        out=slab[:],
                        out_offset=None,
                        in_=ctx_d[:],
                        in_offset=bass.IndirectOffsetOnAxis(
                            ap=idx_t[:, l:l + 1], axis=0),
                    )
                    acc = accs[l % k_bufs]
                    nc.vector.tensor_tensor(out=acc[:], in0=acc[:],
                                            in1=slab[:],
                                            op=mybir.AluOpType.max)
                step = 1
                while step < k_bufs:
                    for k in range(0, k_bufs, 2 * step):
                        if k + step < k_bufs:
                            nc.vector.tensor_tensor(
                                out=accs[k][:], in0=accs[k][:],
                                in1=accs[k + step][:],
                                op=mybir.AluOpType.max)
                    step *= 2
                if acc_bf16:
                    accf = persist.tile([N_SPANS, H], mybir.dt.float32,
                                        tag="accf")
                    nc.vector.tensor_copy(out=accf[:], in_=accs[0][:])
                    nc.sync.dma_start(out=out_d[:], in_=accf[:])
                else:
                    nc.sync.dma_start(out=out_d[:], in_=accs[0][:])
    nc.compile()
    return nc


def _build_split_program(n_steps, n_part, p_split, k_bufs=2, repeat=1,
                         n_slab_bufs=6):
    """Split-span variant: the p_split longest spans (per core, sorted order)
    occupy two partitions each — first half at partition r, second half at
    p_split + r — cutting the accumulate step count from 64 to n_steps.
    Singles live at partitions [2*p_split, n_part). One extra TT combines
    the halves at the end."""
    import concourse.bass as bass
    import concourse.bacc as bacc
    import concourse.mybir as mybir
    import concourse.tile as tile

    nc = bacc.Bacc("TRN2", target_bir_lowering=False, debug=False,
                   num_devices=N_CORES)
    ctx_d = nc.dram_tensor("ctx", [S, H], mybir.dt.float32, kind="ExternalInput")
    idx_d = nc.dram_tensor("idx", [n_part, n_steps], mybir.dt.int32,
                           kind="ExternalInput")
    out_d = nc.dram_tensor("out", [N_SPANS, H], mybir.dt.float32,
                           kind="ExternalOutput")

    with tile.TileContext(nc) as tc:
        with (
            tc.tile_pool(name="persist", bufs=1) as persist,
            tc.tile_pool(name="slabs", bufs=n_slab_bufs) as slabs,
        ):
            idx_t = persist.tile([n_part, n_steps], mybir.dt.int32)
            nc.sync.dma_start(out=idx_t[:], in_=idx_d[:])
            for _ in range(repeat):
                accs = []
                for k in range(k_bufs):
                    acc = persist.tile([n_part, H], mybir.dt.float32,
                                       tag=f"acc{k}")
                    nc.vector.memset(acc[:], -3.0e38)
                    accs.append(acc)
                for l in range(n_steps):
                    slab = slabs.tile([n_part, H], mybir.dt.float32)
                    nc.gpsimd.indirect_dma_start(
                        out=slab[:],
                        out_offset=None,
                        in_=ctx_d[:],
                        in_offset=bass.IndirectOffsetOnAxis(
                            ap=idx_t[:, l:l + 1], axis=0),
                    )
                    acc = accs[l % k_bufs]
                    nc.vector.tensor_tensor(out=acc[:], in0=acc[:],
                                            in1=slab[:],
                                            op=mybir.AluOpType.max)
                step = 1
                while step < k_bufs:
                    for k in range(0, k_bufs, 2 * step):
                        if k + step < k_bufs:
                            nc.vector.tensor_tensor(
                                out=accs[k][:], in0=accs[k][:],
                                in1=accs[k + step][:],
                                op=mybir.AluOpType.max)
                    step *= 2
                if p_split > 0:
                    # second halves live at partition 32 (DVE partition
                    # starts must be 32-aligned; TT inputs must share a
                    # base partition, so bounce through a copy first)
                    halves = persist.tile([32, H], mybir.dt.float32,
                                          tag="halves")
                    nc.vector.tensor_copy(out=halves[0:p_split, :],
                                          in_=accs[0][32:32 + p_split, :])
                    nc.vector.tensor_tensor(
                        out=accs[0][0:p_split, :],
                        in0=accs[0][0:p_split, :],
                        in1=halves[0:p_split, :],
                        op=mybir.AluOpType.max)
                    nc.sync.dma_start(out=out_d[0:32, :],
                                      in_=accs[0][0:32, :])
                    nc.sync.dma_start(
                        out=out_d[32:N_SPANS, :],
                        in_=accs[0][32 + p_split:n_part, :])
                else:
                    nc.sync.dma_start(out=out_d[:], in_=accs[0][:])
    nc.compile()
    return nc


def _make_split_schedule(spans_begin, spans_len):
    """Returns (idx [B, n_part, T], order [B, N], T, n_part, p_split) or None
    if splitting is not beneficial."""
    eff = np.maximum(spans_len, 1)
    order = np.argsort(-eff, axis=1, kind="stable")          # [B, N]
    eff_s = np.take_along_axis(eff, order, axis=1)
    begin_s = np.take_along_axis(spans_begin, order, axis=1)
    max_pairs = 128 - N_SPANS                                 # spare partitions
    half_max = int(-(-int(eff_s.max()) // 2))                 # ceil(maxlen/2)
    best = None
    for T in range(max(half_max, 1), MAX_LEN):
        p_needed = int((eff_s > T).sum(axis=1).max())
        if p_needed <= max_pairs:
            best = (T, p_needed)
            break
    if best is None or best[0] >= MAX_LEN - 2:
        return None
    T, P = best
    if P == 0:
        return None
    if P > 32 - 4:
        return None                                           # need P < 32
    n_part = N_SPANS + P
    t = np.arange(T, dtype=np.int64)[None, None, :]
    idx = np.empty((B, n_part, T), dtype=np.int64)
    # pair spans: sorted ranks [0, P) split in half; first halves at
    # partitions [0, P), second halves at 32-aligned [32, 32+P)
    lenA = -(-eff_s[:, :P] // 2)                              # ceil(eff/2)
    lenB = eff_s[:, :P] - lenA                                # may be 0
    idx[:, 0:P, :] = begin_s[:, :P, None] + np.minimum(t, lenA[:, :, None] - 1)
    start2 = begin_s[:, :P] + lenA
    idx[:, 32:32 + P, :] = np.where(
        lenB[:, :, None] > 0,
        start2[:, :, None] + np.minimum(t, np.maximum(lenB[:, :, None], 1) - 1),
        begin_s[:, :P, None])                                 # dup row if empty
    # singles: ranks [P, 32) at partitions [P, 32); ranks [32, N) at
    # partitions [32+P, n_part).  eff <= T for all of them by construction.
    idx[:, P:32, :] = begin_s[:, P:32, None] + np.minimum(
        t, eff_s[:, P:32, None] - 1)
    idx[:, 32 + P:, :] = begin_s[:, 32:, None] + np.minimum(
        t, eff_s[:, 32:, None] - 1)
    idx = np.clip(idx, 0, S - 1).astype(np.int32)
    return idx, order, T, n_part, P


def _get_program(n_steps=MAX_LEN):
    key = ("v1", n_steps, 2)
    if key not in _cache:
        _cache[key] = _build_program(n_steps, 2)
    return _cache[key]


def _get_split_program(n_steps, n_part, p_split, repeat=1):
    key = ("v2", n_steps, n_part, p_split, repeat)
    if key not in _cache:
        _cache[key] = _build_split_program(n_steps, n_part, p_split,
                                           repeat=repeat)
    return _cache[key]


def _make_indices(spans_begin, spans_len, n_steps=MAX_LEN):
    eff = np.maximum(spans_len, 1)                       # [B, N]
    steps = np.arange(n_steps, dtype=np.int32)           # [L]
    idx = spans_begin[:, :, None] + np.minimum(steps[None, None, :],
                                               eff[:, :, None] - 1)
    return np.clip(idx, 0, S - 1).astype(np.int32)       # [B, N, L]


def kernel(context, spans_begin, spans_len):
    from concourse.bass_utils import run_bass_kernel_spmd

    context = np.ascontiguousarray(context, dtype=np.float32)
    spans_begin = np.asarray(spans_begin, dtype=np.int32)
    spans_len = np.asarray(spans_len, dtype=np.int32)
    assert context.shape == (B, S, H), context.shape
    assert spans_begin.shape == (B, N_SPANS), spans_begin.shape

    sched = _make_split_schedule(spans_begin, spans_len)
    if sched is not None:
        idx, order, n_steps, n_part, p_split = sched
        nc = _get_split_program(n_steps, n_part, p_split)
        in_maps = [{"ctx": context[b], "idx": idx[b]} for b in range(B)]
        res = run_bass_kernel_spmd(nc, in_maps, list(range(N_CORES)))
        out_sorted = np.stack([res.results[b]["out"] for b in range(B)], axis=0)
        out = np.empty_like(out_sorted)
        for b in range(B):
            out[b, order[b]] = out_sorted[b]
        return out.astype(np.float32)

    # fallback: unsplit schedule
    n_steps = int(min(MAX_LEN, max(1, np.maximum(spans_len, 1).max())))
    idx = _make_indices(spans_begin, spans_len, n_steps)
    nc = _get_program(n_steps)
    in_maps = [{"ctx": context[b], "idx": idx[b]} for b in range(B)]
    res = run_bass_kernel_spmd(nc, in_maps, list(range(N_CORES)))
    out = np.stack([res.results[b]["out"] for b in range(B)], axis=0)
    return out.astype(np.float32)



# revision 17
# speedup vs baseline: 1.0499x; 1.0499x over previous
"""Max-pooling over sequence spans — Trainium2 Bass kernel (v3).

Problem: context [B=8, S=4096, H=1024] f32; spans_begin/spans_len [B, 100] i32.
Output [B, 100, H] f32: out[b, n] = max over rows context[b, begin:begin+max(len,1)].

Sharding: pure data-parallel over the batch axis — one batch row per
NeuronCore, 8 cores, no cross-device communication.

The baseline (64 per-step indirect gathers + 64 DVE f32 maxes) saturated
three resources at ~70 us: DMA bytes (26 MB), DVE tensor-tensor time, and
gpsimd SWDGE descriptor generation.  v3 cuts all three:

  * HW indirect DMA applies ONE offset per partition and fills that
    partition's whole destination extent from CONSECUTIVE source rows.
    Span rows are consecutive, so one descriptor fetches WIDTH=4 rows
    (16 KB).  A span of length l needs ceil(l/4) descriptors; the last
    one clamps its base to (begin + l - 4), overlapping already-read rows
    (overlap is max-idempotent).  SWDGE calls: 64 -> ~16.
  * Lanes are sorted by span length (descending), so lanes active in
    round r form a prefix [0, n_r); each round's gather only issues n_r
    descriptors.  DMA traffic drops from 26 MB to ~sum(len)*4KB (~15 MB).
  * The per-step elementwise max is split between DVE and the scalar
    (ACT) engine: ACT converts 3 of 4 slab rows to bf16 (activation
    Copy), DVE runs those maxes in bf16 (2x rate) plus one f32 max per
    round.  Adds ~4e-3 relative error (gate is 2e-2).
  * Spans longer than T are split in two (extra lanes 100..127, <=28);
    partials are combined on the host with np.maximum (free).
  * Short spans (len < 4) can't use 4-row descriptors; <=3 width-1 tail
    rounds cover the lane suffix holding them.  The context is padded
    with four -3e38 rows; garbage lanes in the first K_SLAB rounds fetch
    that pad row, so every slab cell is initialized before first read.

kernel() compiles the Bass program on first call per schedule signature
and caches it for the process lifetime.
"""

import sys
import numpy as np

sys.path.insert(0, "/opt/trn_rl_repo")

B, S, H = 8, 4096, 1024
N_SPANS = 100
MAX_LEN = 64
N_CORES = 8

WIDTH = 4          # rows per main-round descriptor
K_SLAB = 3         # main slab buffers (first K_SLAB rounds gather all lanes)
K_TAIL = 3         # tail slab buffers (must exceed the DMA lookahead of 2)
CONV_J = (1, 2, 3)  # in-round positions converted f32->bf16 on ACT
NEG = -3.0e38
S_PAD = S          # first -inf pad row in the padded context [S+WIDTH, H]

_cache = {}


def _make_schedule(spans_begin, spans_len):
    """Host schedule.  Returns (idx [B,128,R] i32, seg_span [B,128], sched)
    where sched = (R_main, R_tail, n_r tuple, s_tail)."""
    eff = np.maximum(spans_len.astype(np.int64), 1)          # [B, N]
    begin = spans_begin.astype(np.int64)
    n_spare = 128 - N_SPANS

    T = 1
    for b in range(B):
        srt = np.sort(eff[b])[::-1]
        Tb = max(int(-(-srt[0] // 2)), int(srt[n_spare]))
        T = max(T, Tb)
    T = int(min(T, MAX_LEN))

    seg_begin = np.zeros((B, 128), np.int64)
    seg_len = np.zeros((B, 128), np.int64)
    seg_span = np.full((B, 128), -1, np.int64)
    for b in range(B):
        sb = np.zeros(128, np.int64)
        sl = np.zeros(128, np.int64)
        sp = np.full(128, -1, np.int64)
        sb[:N_SPANS] = begin[b]
        sl[:N_SPANS] = eff[b]
        sp[:N_SPANS] = np.arange(N_SPANS)
        nxt = N_SPANS
        for n in np.where(eff[b] > T)[0]:
            la = (eff[b, n] + 1) // 2
            sl[n] = la
            sb[nxt] = begin[b, n] + la
            sl[nxt] = eff[b, n] - la
            sp[nxt] = n
            nxt += 1
        order = np.argsort(-sl, kind="stable")
        seg_begin[b] = sb[order]
        seg_len[b] = sl[order]
        seg_span[b] = sp[order]

    # main rounds: lane p participates in round r iff len_p >= WIDTH and
    # r < ceil(len_p / WIDTH); sorted desc => prefix.
    R_main = int(-(-T // WIDTH))
    rounds = np.arange(R_main)
    part = (seg_len[:, :, None] >= WIDTH) & \
           (rounds[None, None, :] < -(-seg_len[:, :, None] // WIDTH))  # [B,128,R]
    n_r = part.sum(axis=1).max(axis=0)                                  # [R]
    n_r = [128 if r < K_SLAB else int(max(n_r[r], 2)) for r in range(R_main)]

    # tail: short lanes (len < WIDTH) live at the bottom (sorted).  The
    # tail pipeline gathers into a base-0 [n_tail, H] tile (HW indirect
    # DMA requires destination partition base 0); tail offsets for lanes
    # [128 - n_tail, 128) are stored at idx rows [0, n_tail).  Partials
    # are a second output, merged on the host.
    short = seg_len < WIDTH                                             # [B,128]
    if short.any():
        n_short = int(short.sum(axis=1).max())
        n_tail = 32
        while n_tail < n_short:
            n_tail += 32
        n_tail = min(n_tail, 128)
        R_tail = int(max(seg_len[short].max(), 1))  # max short-lane len
    else:
        n_tail, R_tail = 0, 0
    s_tail = 128 - n_tail

    # idx table: main then tail columns.
    R = R_main + R_tail
    idx = np.full((B, 128, R), S_PAD, np.int32)
    for b in range(B):
        ln = seg_len[b]
        bg = seg_begin[b]
        ok = ln >= WIDTH
        base = bg[:, None] + np.minimum(rounds[None, :] * WIDTH,
                                        (ln - WIDTH)[:, None])
        use = ok[:, None] & (rounds[None, :] < -(-ln[:, None] // WIDTH))
        m = np.where(use, base, S_PAD)
        idx[b, :, :R_main] = np.clip(m, 0, S_PAD)
        if R_tail:
            t = np.arange(R_tail)
            tl = bg[s_tail:, None] + np.minimum(
                t[None, :], np.maximum(ln[s_tail:], 1)[:, None] - 1)
            idx[b, :n_tail, R_main:] = np.clip(tl, 0, S - 1)
    sched = (R_main, R_tail, tuple(n_r), n_tail)
    return idx, seg_span, sched


def _build_v3(sched, repeat=1):
    """Build + compile the per-core SPMD program for a given schedule."""
    import concourse.bass as bass
    import concourse.bacc as bacc
    import concourse.mybir as mybir
    import concourse.tile as tile

    R_main, R_tail, n_r, n_tail = sched
    R = R_main + R_tail
    nc = bacc.Bacc("TRN2", target_bir_lowering=False, debug=False,
                   num_devices=N_CORES)
    ctx_d = nc.dram_tensor("ctx", [S + WIDTH, H], mybir.dt.float32,
                           kind="ExternalInput")
    idx_d = nc.dram_tensor("idx", [128, R], mybir.dt.int32, kind="ExternalInput")
    out_d = nc.dram_tensor("out", [128, H], mybir.dt.float32,
                           kind="ExternalOutput")
    out2_d = nc.dram_tensor("out2", [max(n_tail, 1), H], mybir.dt.float32,
                            kind="ExternalOutput") if R_tail else None

    n_conv = len(CONV_J)
    with tile.TileContext(nc) as tc:
        with tc.tile_pool(name="persist", bufs=1) as persist:
            idx_t = persist.tile([128, R], mybir.dt.int32)
            nc.sync.dma_start(out=idx_t[:], in_=idx_d[:])
            slab_bufs = [persist.tile([128, WIDTH * H], mybir.dt.float32,
                                      name=f"slab{k}", tag=f"slab{k}")
                         for k in range(K_SLAB)]
            cslab_bufs = [persist.tile([128, max(n_conv, 1) * H],
                                       mybir.dt.bfloat16,
                                       name=f"cslab{k}", tag=f"cslab{k}")
                          for k in range(K_SLAB)] if n_conv else []
            tail_bufs = [persist.tile([n_tail, H], mybir.dt.float32,
                                      name=f"tslab{k}", tag=f"tslab{k}")
                         for k in range(K_TAIL)] if R_tail else []
            for _ in range(repeat):
                acc0 = persist.tile([128, H], mybir.dt.float32, tag="acc0")
                acc1 = persist.tile([128, H], mybir.dt.float32, tag="acc1")
                accb0 = persist.tile([128, H], mybir.dt.bfloat16, tag="accb0")
                accb1 = persist.tile([128, H], mybir.dt.bfloat16, tag="accb1")
                nc.gpsimd.memset(acc0[:], NEG)
                nc.gpsimd.memset(acc1[:], NEG)
                nc.gpsimd.memset(accb0[:], NEG)
                nc.gpsimd.memset(accb1[:], NEG)
                if R_tail:
                    acc_t = persist.tile([n_tail, H], mybir.dt.float32,
                                         tag="acc_t")
                    nc.gpsimd.memset(acc_t[:], NEG)

                slab_tiles = {}

                def emit_dma(r):
                    if r < R_main:
                        n = n_r[r]
                        slab = slab_bufs[r % K_SLAB]
                        nc.gpsimd.indirect_dma_start(
                            out=slab[0:n, :],
                            out_offset=None,
                            in_=ctx_d[:],
                            in_offset=bass.IndirectOffsetOnAxis(
                                ap=idx_t[0:n, r:r + 1], axis=0),
                        )
                    else:
                        slab = tail_bufs[(r - R_main) % K_TAIL]
                        nc.gpsimd.indirect_dma_start(
                            out=slab[:],
                            out_offset=None,
                            in_=ctx_d[:],
                            in_offset=bass.IndirectOffsetOnAxis(
                                ap=idx_t[0:n_tail, r:r + 1], axis=0),
                        )
                    slab_tiles[r] = slab

                emit_dma(0)
                if R > 1:
                    emit_dma(1)
                dve_i = 0
                bf_i = 0
                for r in range(R):
                    if r + 2 < R:
                        emit_dma(r + 2)
                    slab = slab_tiles.pop(r)
                    if r < R_main:
                        cslab = cslab_bufs[r % K_SLAB] if n_conv else None
                        for j in range(WIDTH):
                            sl = slab[:, j * H:(j + 1) * H]
                            if j in CONV_J:
                                m = CONV_J.index(j)
                                cs = cslab[:, m * H:(m + 1) * H]
                                nc.scalar.activation(
                                    out=cs, in_=sl,
                                    func=mybir.ActivationFunctionType.Copy)
                                a = (accb0, accb1)[bf_i % 2]
                                bf_i += 1
                                nc.vector.tensor_tensor(
                                    out=a[:], in0=a[:], in1=cs,
                                    op=mybir.AluOpType.max)
                            else:
                                a = (acc0, acc1)[dve_i % 2]
                                dve_i += 1
                                nc.vector.tensor_tensor(
                                    out=a[:], in0=a[:], in1=sl,
                                    op=mybir.AluOpType.max)
                    else:
                        nc.vector.tensor_tensor(
                            out=acc_t[:], in0=acc_t[:], in1=slab[:],
                            op=mybir.AluOpType.max)
                nc.vector.tensor_tensor(out=accb0[:], in0=accb0[:],
                                        in1=accb1[:], op=mybir.AluOpType.max)
                nc.vector.tensor_tensor(out=acc0[:], in0=acc0[:], in1=acc1[:],
                                        op=mybir.AluOpType.max)
                nc.vector.tensor_tensor(out=acc0[:], in0=acc0[:], in1=accb0[:],
                                        op=mybir.AluOpType.max)
                nc.sync.dma_start(out=out_d[:], in_=acc0[:])
                if R_tail:
                    nc.sync.dma_start(out=out2_d[:], in_=acc_t[:])
    nc.compile()
    return nc


def _get_v3(sched, repeat=1):
    key = ("v3", sched, repeat, CONV_J)
    if key not in _cache:
        _cache[key] = _build_v3(sched, repeat=repeat)
    return _cache[key]


def _pad_ctx(context):
    pad = np.full((B, WIDTH, H), NEG, dtype=np.float32)
    return np.concatenate([context, pad], axis=1)   # [B, S+WIDTH, H]


def _combine_host(out_lanes, seg_span):
    """out_lanes [B,128,H] sorted-lane partials -> [B,N_SPANS,H]."""
    res = np.full((B, N_SPANS, H), -np.inf, dtype=np.float32)
    for b in range(B):
        valid = seg_span[b] >= 0
        np.maximum.at(res[b], seg_span[b][valid], out_lanes[b][valid])
    return res


def kernel(context, spans_begin, spans_len):
    from concourse.bass_utils import run_bass_kernel_spmd

    context = np.ascontiguousarray(context, dtype=np.float32)
    spans_begin = np.asarray(spans_begin, dtype=np.int32)
    spans_len = np.asarray(spans_len, dtype=np.int32)
    assert context.shape == (B, S, H), context.shape
    assert spans_begin.shape == (B, N_SPANS), spans_begin.shape

    idx, seg_span, sched = _make_schedule(spans_begin, spans_len)
    R_main, R_tail, n_r, n_tail = sched
    nc = _get_v3(sched)
    ctx_pad = _pad_ctx(context)
    in_maps = [{"ctx": ctx_pad[b], "idx": idx[b]} for b in range(B)]
    res = run_bass_kernel_spmd(nc, in_maps, list(range(N_CORES)))
    out_lanes = np.stack([res.results[b]["out"] for b in range(B)], axis=0)
    if R_tail:
        for b in range(B):
            np.maximum(out_lanes[b, 128 - n_tail:],
                       res.results[b]["out2"],
                       out=out_lanes[b, 128 - n_tail:])
    return _combine_host(out_lanes, seg_span).astype(np.float32)
